# revision 1
# baseline (speedup 1.0000x reference)
"""ACE/SPADE block (nn_ACE_48808008352089) as a distributed Bass kernel on
8 TRN2 NeuronCores.

Sharding: data-parallel over (batch, image-half): core c handles batch
b = c // 2, rows [128*(c%2), 128*(c%2)+128).  BatchNorm is SyncBN via a
[128,2] AllReduce of per-core (sum, sumsq).  Small weights replicated.

Structure (per core):
  phase A : xn = x + nv*noise (noise term via K=1 TensorE outer product),
            per-channel sum/sumsq accumulated on the fly, xn spilled to a
            DRAM scratch.  AllReduce of [128,2] partials -> mean/rstd.
  mu/A    : per-label FC mu = relu(Wfc@sc+bfc) and folded style tables
            A = mu @ (blend-scaled w_cg/w_cb) on TensorE.
  pass 1  : label one-hot S (9 shifts x 19 labels stacked on 114+57
            partitions; broadcast-DMA + DVE is_equal, one 32-row quarter
            at a time, double buffered), actv = relu(w_sh-conv(S)+b_sh),
            gamma/beta = spade-conv(actv) + A-conv(S) accumulated in PSUM
            (weight-stationary over 4 row-pairs), staged to SBUF in bf16.
  pass 2  : norm = (xn-mean)*rstd; out = norm*(gamma+gbias)+(beta+bbias).
            Only pass 2 depends on the BN stats, so TensorE never stalls
            on the collective.
"""
import numpy as np
import ml_dtypes

from concourse import bacc, tile, mybir
from concourse.bass_utils import run_bass_kernel_spmd

BF16 = ml_dtypes.bfloat16

B, C, H, W, L, S = 4, 128, 256, 256, 19, 512
NCORES = 8
NROWS = 128                 # output rows per core
WP = W + 2                  # zero-padded width
NPIX = NROWS * W            # 32768
BLK = 8                     # output rows per block
NBLK = NROWS // BLK         # 16
QROWS = 16                  # rows per one-hot quarter
SQP = (QROWS + 2) * WP      # one-hot pixels per quarter (34*258)
LABN = (NROWS + 4) * WP + 2  # guarded labels buffer len (132*258+2)
ACH = 2048                  # phase-A chunk pixels
NCH = NPIX // ACH           # 8 chunks
OFFS9 = [(dy, dx) for dy in (-1, 0, 1) for dx in (-1, 0, 1)]
NTOT = float(B * H * W)
EPS = 1e-5

F32 = mybir.dt.float32
BF = mybir.dt.bfloat16
I8 = mybir.dt.int8
AL = mybir.AluOpType
AF = mybir.ActivationFunctionType

_cache = {}


def build_nc():
    nc = bacc.Bacc("TRN2", target_bir_lowering=False, debug=False,
                   num_devices=NCORES, num_swdge_queues=4)

    def inp(name, shape, dt):
        return nc.dram_tensor(name, shape, dt, kind="ExternalInput")

    x_d = inp("x", [C, NPIX], F32)
    noi_d = inp("noiseT", [NPIX], BF)
    lab_d = inp("labels", [LABN], I8)
    wfc_d = inp("wfc", [128, 304 * 128], BF)     # [dp, (j, dc, mc, o)]
    sct_d = inp("sct", [128, 4 * L], BF)
    bfct_d = inp("bfct", [128, 4 * L], F32)
    wcg_d = inp("wcg", [128, 4 * 1152], BF)
    wcb_d = inp("wcb", [128, 4 * 1152], BF)
    wsh1_d = inp("wsh1", [114, 128], BF)
    wsh2_d = inp("wsh2", [57, 128], BF)
    wg_d = inp("wg", [128, 9 * 128], BF)
    wb_d = inp("wb", [128, 9 * 128], BF)
    nv_d = inp("nv", [C, 1], F32)
    nvt_d = inp("nvT", [1, C], BF)
    gbias_d = inp("gbias", [C, 1], F32)
    bbias_d = inp("bbias", [C, 1], F32)
    bshm_d = inp("bshm", [C, NROWS + 2], F32)
    jc114_d = inp("jc114", [114, 1], F32)
    jc57_d = inp("jc57", [57, 1], F32)
    out_d = nc.dram_tensor("out", [C, NPIX], F32, kind="ExternalOutput")

    with tile.TileContext(nc) as tc:
        with tc.tile_pool(name="const", bufs=1) as cp, \
             tc.tile_pool(name="dram", bufs=1, space="DRAM") as dramp, \
             tc.tile_pool(name="xp", bufs=3) as xp, \
             tc.tile_pool(name="noi", bufs=2) as noip, \
             tc.tile_pool(name="rp", bufs=1) as rp, \
             tc.tile_pool(name="mw", bufs=2) as mw, \
             tc.tile_pool(name="ps", bufs=2, space="PSUM") as psp, \
             tc.tile_pool(name="psgb", bufs=4, space="PSUM") as psgb, \
             tc.tile_pool(name="psn", bufs=2, space="PSUM") as psn:

            # ---------- tiny consts (qACT) ----------
            def cdma(t, src):
                nc.scalar.dma_start(out=t[:], in_=src)
                return t
            nv = cdma(cp.tile([C, 1], F32, name="nv", tag="nv"), nv_d[:])
            nvt = cdma(cp.tile([1, C], BF, name="nvt", tag="nvt"), nvt_d[:])
            gbias = cdma(cp.tile([C, 1], F32, name="gbias", tag="gbias"), gbias_d[:])
            bbias = cdma(cp.tile([C, 1], F32, name="bbias", tag="bbias"), bbias_d[:])
            bshm = cdma(cp.tile([C, NROWS + 2], F32, name="bshm", tag="bshm"), bshm_d[:])
            jc114 = cdma(cp.tile([114, 1], F32, name="jc114", tag="jc114"), jc114_d[:])
            jc57 = cdma(cp.tile([57, 1], F32, name="jc57", tag="jc57"), jc57_d[:])
            wsh1 = cdma(cp.tile([114, 128], BF, name="wsh1", tag="wsh1"), wsh1_d[:])
            wsh2 = cdma(cp.tile([57, 128], BF, name="wsh2", tag="wsh2"), wsh2_d[:])
            wg = cdma(cp.tile([128, 9 * 128], BF, name="wg", tag="wg"), wg_d[:])
            wb = cdma(cp.tile([128, 9 * 128], BF, name="wb", tag="wb"), wb_d[:])
            sct = cdma(cp.tile([128, 4 * L], BF, name="sct", tag="sct"), sct_d[:])
            bfct = cdma(cp.tile([128, 4 * L], F32, name="bfct", tag="bfct"), bfct_d[:])

            ag1 = cp.tile([114, 128], BF)
            ag2 = cp.tile([57, 128], BF)
            ab1 = cp.tile([114, 128], BF)
            ab2 = cp.tile([57, 128], BF)
            sums_x = cp.tile([C, NCH * 4], F32)
            sums_q = cp.tile([C, NCH * 4], F32)
            stats2 = cp.tile([C, 2], F32)
            stats_g = cp.tile([C, 2], F32)
            m_t = cp.tile([C, 1], F32)
            e_t = cp.tile([C, 1], F32)
            nvar = cp.tile([C, 1], F32)
            varp = cp.tile([C, 1], F32)
            sqv = cp.tile([C, 1], F32)
            rstd = cp.tile([C, 1], F32)
            nmr = cp.tile([C, 1], F32)

            s1a = cp.tile([114, QROWS + 2, WP], BF)
            s2a = cp.tile([57, QROWS + 2, WP], BF)
            s1b = cp.tile([114, QROWS + 2, WP], BF)
            s2b = cp.tile([57, QROWS + 2, WP], BF)
            s_sets = [(s1a, s2a), (s1b, s2b)]

            xn_sb = cp.tile([C, NPIX], BF)

            def build_s(q):
                """label one-hot quarter q (reps on qACT, is_equal on DVE)."""
                s1t, s2t = s_sets[q % 2]
                rep1 = rp.tile([114, QROWS + 2, WP], I8, tag="rep",
                               name=f"rep1_{q}")
                for g in range(6):
                    dy, dx = OFFS9[g]
                    base = (QROWS * q + 1 + dy) * WP + dx + 1
                    nc.sync.dma_start(
                        out=rep1[g * L:(g + 1) * L, :, :],
                        in_=lab_d[base:base + SQP].partition_broadcast(L))
                nc.vector.tensor_scalar(s1t[:], rep1[:], jc114[:], None,
                                        AL.is_equal)
                rep2 = rp.tile([57, QROWS + 2, WP], I8, tag="rep",
                               name=f"rep2_{q}")
                for g in range(6, 9):
                    dy, dx = OFFS9[g]
                    base = (QROWS * q + 1 + dy) * WP + dx + 1
                    nc.sync.dma_start(
                        out=rep2[(g - 6) * L:(g - 5) * L, :, :],
                        in_=lab_d[base:base + SQP].partition_broadcast(L))
                nc.vector.tensor_scalar(s2t[:], rep2[:], jc57[:], None,
                                        AL.is_equal)

            # wfc split across both HW DMA queues (j-major layout)
            wfp = tc.alloc_tile_pool(name="wfp", bufs=1)

            build_s(0)
            wcg = wfp.tile([128, 4 * 1152], BF)
            nc.scalar.dma_start(out=wcg[:, :2304], in_=wcg_d[:, :2304])
            nc.scalar.dma_start(out=wcg[:, 2304:], in_=wcg_d[:, 2304:])
            wcb = wfp.tile([128, 4 * 1152], BF)
            nc.scalar.dma_start(out=wcb[:, :2304], in_=wcb_d[:, :2304])
            nc.scalar.dma_start(out=wcb[:, 2304:], in_=wcb_d[:, 2304:])
            build_s(1)

            # ---------- phase A: xn + stats ----------
            for ci in range(NCH):
                npc = noip.tile([1, ACH], BF, tag="noi", name=f"noi{ci}")
                nc.sync.dma_start(out=npc[:],
                                  in_=noi_d[ci * ACH:(ci + 1) * ACH]
                                  .unsqueeze(0))
                xb = xp.tile([C, ACH], F32, tag="x", name=f"xA{ci}")
                for pi in range(4):
                    nc.sync.dma_start(
                        out=xb[32 * pi:32 * (pi + 1), :],
                        in_=x_d[32 * pi:32 * (pi + 1),
                                ci * ACH:(ci + 1) * ACH])
                for i in range(ACH // 512):
                    col = ci * 4 + i
                    p0 = ci * ACH + i * 512
                    pn = psn.tile([C, 512], F32, tag="pn", name=f"pnA{ci}_{i}")
                    nc.tensor.matmul(pn[:], lhsT=nvt[:],
                                     rhs=npc[:, i * 512:(i + 1) * 512],
                                     start=True, stop=True)
                    nc.vector.scalar_tensor_tensor(
                        out=xn_sb[:, p0:p0 + 512], in0=pn[:],
                        scalar=1.0, in1=xb[:, i * 512:(i + 1) * 512],
                        op0=AL.mult, op1=AL.add,
                        accum_out=sums_x[:, col:col + 1])
                    sq = psn.tile([C, 512], F32, tag="pn", name=f"sqA{ci}_{i}")
                    nc.scalar.activation(
                        sq[:], xn_sb[:, p0:p0 + 512], AF.Square,
                        accum_out=sums_q[:, col:col + 1])

            nc.vector.tensor_reduce(out=stats2[:, 0:1], in_=sums_x[:],
                                    axis=mybir.AxisListType.X, op=AL.add)
            nc.vector.tensor_reduce(out=stats2[:, 1:2], in_=sums_q[:],
                                    axis=mybir.AxisListType.X, op=AL.add)
            stat_in = dramp.tile([C, 2], F32)
            stat_out = dramp.tile([C, 2], F32)
            nc.sync.dma_start(out=stat_in[:], in_=stats2[:])
            nc.gpsimd.collective_compute(
                "AllReduce", AL.add, replica_groups=[list(range(NCORES))],
                ins=[stat_in.opt()], outs=[stat_out.opt()])
            nc.sync.dma_start(out=stats_g[:], in_=stat_out[:])

            # ---------- mu + A tables (wfc loaded in two halves) ----------
            mu_sb = cp.tile([128, 4 * L], BF)
            mu_psum = [psn.tile([C, 512], F32, tag="pn", name=f"pnmu{m}")
                       for m in range(2)]
            for hf, (j0, j1) in enumerate([(0, 10), (10, L)]):
                ncols = (j1 - j0) * 2048
                wfc_h = wfp.tile([128, 20 * 1024], BF, tag="wf",
                                 name=f"wfc{hf}")
                for wi in range(4):
                    c0 = wi * ncols // 4
                    c1 = (wi + 1) * ncols // 4
                    nc.scalar.dma_start(out=wfc_h[:, c0:c1],
                                        in_=wfc_d[:, j0 * 2048 + c0:
                                                  j0 * 2048 + c1])
                pnmu = mu_psum[hf]
                for j in range(j0, j1):
                    for mc in range(4):
                        for dc in range(4):
                            off = (j - j0) * 2048 + dc * 512 + mc * 128
                            nc.tensor.matmul(
                                pnmu[:, (j - j0) * 4 + mc:
                                     (j - j0) * 4 + mc + 1],
                                lhsT=wfc_h[:, off:off + 128],
                                rhs=sct[:, dc * L + j:dc * L + j + 1],
                                start=(dc == 0), stop=(dc == 3))
            # pnmu halves hold mu columns interleaved (j,mc); rearrange into
            # mu_sb [128, (mc, j)] via per-column adds
            for mc in range(4):
                mtmp = mw.tile([128, L], F32, tag="mtmp", name=f"mt{mc}")
                for j in range(L):
                    hf = 0 if j < 10 else 1
                    jj = j - (0 if j < 10 else 10)
                    nc.vector.tensor_add(
                        mtmp[:, j:j + 1],
                        mu_psum[hf][:, jj * 4 + mc:jj * 4 + mc + 1],
                        bfct[:, mc * L + j:mc * L + j + 1])
                nc.scalar.activation(mu_sb[:, mc * L:(mc + 1) * L],
                                     mtmp[:], AF.Relu)
            for tbl, (wsb, a1, a2) in enumerate(
                    [(wcg, ag1, ag2), (wcb, ab1, ab2)]):
                for g in range(9):
                    aps = psp.tile([C, WP], F32, tag="pa", name=f"aps{tbl}{g}")
                    for mc in range(4):
                        nc.tensor.matmul(
                            aps[0:L, 0:128],
                            lhsT=mu_sb[:, mc * L:(mc + 1) * L],
                            rhs=wsb[:, mc * 1152 + g * 128:
                                    mc * 1152 + (g + 1) * 128],
                            start=(mc == 0), stop=(mc == 3))
                    atmp = mw.tile([L, 128], BF, tag="atmp",
                                   name=f"at{tbl}{g}")
                    nc.scalar.copy(atmp[:], aps[0:L, 0:128])
                    dst = (a1 if g < 6 else a2)
                    gg = g if g < 6 else g - 6
                    nc.scalar.dma_start(out=dst[gg * L:(gg + 1) * L, :],
                                        in_=atmp[:])

            wfp.release()
            gsp = tc.alloc_tile_pool(name="gsp", bufs=4)
            ob = tc.alloc_tile_pool(name="ob", bufs=2)
            avp = tc.alloc_tile_pool(name="avp", bufs=1)
            actv_a = avp.tile([C, BLK + 2, WP], BF)
            actv_b = avp.tile([C, BLK + 2, WP], BF)
            actv_bufs = [actv_a, actv_b]
            for ab_ in actv_bufs:
                nc.vector.memset(ab_[:, :, 0:1], 0.0)
                nc.vector.memset(ab_[:, :, WP - 1:WP], 0.0)
            # ---------- pass 1: convs -> gstage/bstage (PE-side) ----------
            NP2 = BLK // 2
            gs_tiles, bs_tiles = [], []
            for kb in range(NBLK):
                q = kb // (QROWS // BLK)
                r0 = kb * BLK
                if kb % (QROWS // BLK) == 0 and q >= 2:
                    build_s(q)
                s1, s2 = s_sets[q % 2]

                def s_row(s3, r):
                    return s3[:, r - QROWS * q + 1, :]

                def s_pair(s3, r):
                    lr = r - QROWS * q + 1
                    return s3[:, lr:lr + 2, 1:W + 1]

                actv = actv_bufs[kb % 2]
                for ir in range(BLK + 2):
                    ar = r0 - 1 + ir
                    pa = psp.tile([C, WP], F32, tag="pa", name=f"pa{kb}_{ir}")
                    nc.tensor.matmul(pa[:], lhsT=wsh1[:], rhs=s_row(s1, ar),
                                     start=True, stop=False)
                    nc.tensor.matmul(pa[:], lhsT=wsh2[:], rhs=s_row(s2, ar),
                                     start=False, stop=True)
                    nc.scalar.activation(actv[:, ir, 1:W + 1], pa[:, 1:W + 1],
                                         AF.Relu, bias=bshm[:, ar + 1:ar + 2])

                gstage = gsp.tile([C, BLK * W], BF, tag="gs", name=f"gs{kb}")
                bstage = gsp.tile([C, BLK * W], BF, tag="bs", name=f"bs{kb}")
                gs_tiles.append(gstage)
                bs_tiles.append(bstage)
                for wsp, a1t, a2t, stage, snm in (
                        (wg, ag1, ag2, gstage, "g"),
                        (wb, ab1, ab2, bstage, "b")):
                    pps = [psgb.tile([C, 2 * W], F32, tag="pgb",
                                     name=f"p{snm}{kb}_{i}")
                           for i in range(NP2)]
                    for g, (dy, dx) in enumerate(OFFS9):
                        for i in range(NP2):
                            nc.tensor.matmul(
                                pps[i][:],
                                lhsT=wsp[:, g * 128:(g + 1) * 128],
                                rhs=actv[:, 2 * i + 1 + dy:2 * i + 3 + dy,
                                         1 + dx:W + 1 + dx],
                                start=(g == 0), stop=False)
                    for i in range(NP2):
                        nc.tensor.matmul(pps[i][:], lhsT=a1t[:],
                                         rhs=s_pair(s1, r0 + 2 * i),
                                         start=False, stop=False)
                    for i in range(NP2):
                        nc.tensor.matmul(pps[i][:], lhsT=a2t[:],
                                         rhs=s_pair(s2, r0 + 2 * i),
                                         start=False, stop=True)
                    for i in range(NP2):
                        nc.scalar.copy(stage[:, 2 * i * W:(2 * i + 2) * W],
                                       pps[i][:])

            # ---------- stats finalize (DVE; waits on the AllReduce) ------
            nc.vector.tensor_scalar_mul(m_t[:], stats_g[:, 0:1], 1.0 / NTOT)
            nc.vector.tensor_scalar_mul(e_t[:], stats_g[:, 1:2], 1.0 / NTOT)
            nc.vector.scalar_tensor_tensor(
                out=nvar[:], in0=m_t[:], scalar=m_t[:], in1=e_t[:],
                op0=AL.mult, op1=AL.subtract)
            nc.vector.tensor_scalar(varp[:], nvar[:], -1.0, EPS,
                                    AL.mult, AL.add)
            nc.scalar.activation(sqv[:], varp[:], AF.Sqrt)
            nc.vector.reciprocal(rstd[:], sqv[:])
            nc.vector.scalar_tensor_tensor(
                out=nmr[:], in0=m_t[:], scalar=-1.0, in1=rstd[:],
                op0=AL.mult, op1=AL.mult)

            # ---------- pass 2: norm + blend + output ---------------------
            for kb in range(NBLK):
                r0 = kb * BLK
                xb = xn_sb[:, r0 * W:(r0 + BLK) * W]
                nc.vector.tensor_scalar(xb, xb, rstd[:], nmr[:],
                                        AL.mult, AL.add)
                ot = ob.tile([C, BLK * W], F32, tag="ot", name=f"ot{kb}")
                nc.vector.scalar_tensor_tensor(
                    out=ot[:], in0=gs_tiles[kb][:], scalar=gbias[:],
                    in1=xb, op0=AL.add, op1=AL.mult)
                nc.vector.scalar_tensor_tensor(
                    out=ot[:], in0=bs_tiles[kb][:], scalar=bbias[:],
                    in1=ot[:], op0=AL.add, op1=AL.add)
                nc.gpsimd.dma_start(out=out_d[:, r0 * W:(r0 + BLK) * W],
                                    in_=ot[:])
            avp.release()
            ob.release()
            gsp.release()
    nc.compile()
    return nc


def _prep_shared(inputs):
    """Host-side weight layout prep (replicated to all cores)."""
    gb = np.asarray(inputs["g_blend"], np.float32).reshape(-1)[0]
    bb = np.asarray(inputs["b_blend"], np.float32).reshape(-1)[0]
    ga = 1.0 / (1.0 + np.exp(-gb))
    ba = 1.0 / (1.0 + np.exp(-bb))
    w_sh = np.asarray(inputs["w_sh"], np.float32)
    w_g = np.asarray(inputs["w_g"], np.float32)
    w_b = np.asarray(inputs["w_b"], np.float32)
    w_cg = np.asarray(inputs["w_cg"], np.float32)
    w_cb = np.asarray(inputs["w_cb"], np.float32)
    Wfc = np.asarray(inputs["Wfc"], np.float32)
    bfc = np.asarray(inputs["bfc"], np.float32)
    b_sh = np.asarray(inputs["b_sh"], np.float32)
    b_g = np.asarray(inputs["b_g"], np.float32)
    b_b = np.asarray(inputs["b_b"], np.float32)
    b_cg = np.asarray(inputs["b_cg"], np.float32)
    b_cb = np.asarray(inputs["b_cb"], np.float32)
    nv = np.asarray(inputs["noise_var"], np.float32)

    sh = {}
    # w_sh [o, j, 3, 3] -> [(dy,dx,j), o] stacked
    wshst = np.ascontiguousarray(
        w_sh.transpose(2, 3, 1, 0).reshape(9 * L, 128)).astype(BF16)
    sh["wsh1"] = np.ascontiguousarray(wshst[:6 * L])
    sh["wsh2"] = np.ascontiguousarray(wshst[6 * L:])
    # w_g/w_b [o, c, 3, 3] -> [c, (g, o)] scaled
    sh["wg"] = np.ascontiguousarray(
        ((1 - ga) * w_g).transpose(1, 2, 3, 0).reshape(128, 9 * 128)).astype(BF16)
    sh["wb"] = np.ascontiguousarray(
        ((1 - ba) * w_b).transpose(1, 2, 3, 0).reshape(128, 9 * 128)).astype(BF16)

    # w_cg/w_cb [o, c(512), 3, 3] -> [128, (cc, g, o)] scaled
    def cvt_cw(wt, scale):
        a = (scale * wt).transpose(1, 2, 3, 0).reshape(512, 9 * 128)
        a = a.reshape(4, 128, 9 * 128).transpose(1, 0, 2).reshape(128, 4 * 1152)
        return np.ascontiguousarray(a).astype(BF16)
    sh["wcg"] = cvt_cw(w_cg, ga)
    sh["wcb"] = cvt_cw(w_cb, ba)
    # Wfc [j, o, d] -> [128(dp), (j, dc, mc, o)]  (j-major for split load)
    f = Wfc.transpose(0, 2, 1).reshape(L, 4, 128, 4, 128)
    f = f.transpose(0, 1, 3, 2, 4)            # [j, dc, mc, dp, o]
    f = f.transpose(3, 0, 1, 2, 4).reshape(128, 304 * 128)
    sh["wfc"] = np.ascontiguousarray(f).astype(BF16)
    # bfc [j, d] -> bfct [128, (mc, j)]
    bf_t = bfc.T.reshape(4, 128, L).transpose(1, 0, 2).reshape(128, 4 * L)
    sh["bfct"] = np.ascontiguousarray(bf_t).astype(np.float32)
    sh["nv"] = np.ascontiguousarray(nv.reshape(C, 1))
    sh["nvT"] = np.ascontiguousarray(nv.reshape(1, C)).astype(BF16)
    sh["gbias"] = np.ascontiguousarray(
        (1.0 + ga * b_cg + (1 - ga) * b_g).reshape(C, 1)).astype(np.float32)
    sh["bbias"] = np.ascontiguousarray(
        (ba * b_cb + (1 - ba) * b_b).reshape(C, 1)).astype(np.float32)
    sh["jc114"] = np.tile(np.arange(L, dtype=np.float32), 6)[:, None].copy()
    sh["jc57"] = np.tile(np.arange(L, dtype=np.float32), 3)[:, None].copy()
    sh["_b_sh"] = b_sh
    return sh


def kernel(**inputs):
    if "nc" not in _cache:
        _cache["nc"] = build_nc()
    nc = _cache["nc"]

    x = np.asarray(inputs["x"], np.float32)
    labels = np.asarray(inputs["labels"]).astype(np.int64)
    noise = np.asarray(inputs["noise"], np.float32)
    style = np.asarray(inputs["style_codes"], np.float32)
    sh = _prep_shared(inputs)
    b_sh = sh.pop("_b_sh")

    in_maps = []
    for c in range(NCORES):
        b, half = c // 2, c % 2
        h0 = half * NROWS
        m = dict(sh)
        m["x"] = np.ascontiguousarray(
            x[b, :, h0:h0 + NROWS, :]).reshape(C, NPIX)
        # noise [B, W, H, 1]: added[c,h,w] = noise[b,w,h]*nv[c]
        m["noiseT"] = np.ascontiguousarray(
            noise[b, :, h0:h0 + NROWS, 0].T).reshape(NPIX).astype(BF16)
        # guarded, padded labels (int8, -1 outside image)
        lab = np.full((NROWS + 4, WP), -1, np.int8)
        lo, hi = max(0, h0 - 2), min(H, h0 + NROWS + 2)
        lab[lo - (h0 - 2):hi - (h0 - 2), 1:W + 1] = labels[b, lo:hi, :]
        g = np.full(LABN, -1, np.int8)
        g[1:1 + (NROWS + 4) * WP] = lab.reshape(-1)
        m["labels"] = g
        # style codes transposed [128, (dc, j)]
        sct = style[b].T.reshape(4, 128, L).transpose(1, 0, 2).reshape(128, 4 * L)
        m["sct"] = np.ascontiguousarray(sct).astype(BF16)
        # b_sh masked per actv row (zero outside image)
        rows = h0 + np.arange(-1, NROWS + 1)
        mask = ((rows >= 0) & (rows < H)).astype(np.float32)
        m["bshm"] = np.ascontiguousarray(b_sh[:, None] * mask[None, :])
        in_maps.append(m)

    res = run_bass_kernel_spmd(nc, in_maps, core_ids=list(range(NCORES)),
                               **_cache.get("run_kwargs", {}))
    _cache["last_result"] = res

    out = np.empty((B, C, H, W), np.float32)
    for c in range(NCORES):
        b, half = c // 2, c % 2
        h0 = half * NROWS
        out[b, :, h0:h0 + NROWS, :] = res.results[c]["out"].reshape(C, NROWS, W)
    return out

